# revision 40
# baseline (speedup 1.0000x reference)
"""Trainium2 Bass kernel for nn_MeshNodeBlock (GNN message passing block).

reference semantics:
    agg = segment_sum(edge_features, src_indices, N)        # scatter-add
    x   = concat([node_features, agg], -1)
    h   = silu(x @ W1 + b1)
    y   = h @ W2 + b2
    y   = layer_norm(y) * gamma + beta
    out = y + node_features

Strategy (8 NeuronCores, SPMD, one NEFF):
  * Host snake-deals nodes by degree into 800 bins (8 cores x 100 tiles) of
    128 slots each, so every tile receives ~750 edges = exactly 6 chunks of
    128 (a contiguous partition needs 7). Each chunk ships bf16 edge
    features (256 B/slot) + fp8 one-hot (128 B/slot).
  * Device works fully in transposed space (features on partitions, nodes on
    free dim). Per 128-node tile the scatter-add is ci PE matmuls
    aggT += edge_chunk.T @ onehot into the group's [128,512] PSUM tile.
  * MLP consumes aggT/nodeT directly: layer 1 -> silu(+b1) on the scalar
    engine, layer 2 -> yT. xta copy on scalar engine; y (+b2) and y^2 on
    the vector engine (y^2 from SBUF, 2x mode).
  * LayerNorm stats via ONCB matmuls (rows of a shared PSUM bank); block
    phase2 computes rstd (ln/exp set) and D = mu*rstd, writes [D|rstd] rows
    to a DRAM bounce; phase3 prefetches them partition-broadcast in one
    large DMA per 7-group sub-block (not 128 tiny replicated reads per
    group) and normalizes in 3-4 fused DVE ops
    (y*rstd - D)*gamma + beta + node, interleaved with next block's work.
  * Output written transposed in bf16; host inverts the node permutation.
"""

import functools
from contextlib import ExitStack

import numpy as np
import ml_dtypes

import concourse.bass as bass
import concourse.tile as tile
from concourse import bacc, mybir
from concourse import bass_utils

BF16 = ml_dtypes.bfloat16
FP8 = ml_dtypes.float8_e4m3

N_NODES = 100000
D = 128
N_CORES = 8
P = 128
GROUP = 512
N_GROUPS = 25
TILES_PER_CORE = 100
NODES_PER_CORE = N_GROUPS * GROUP   # 12800
NBINS = N_CORES * TILES_PER_CORE    # 800
BLOCKS = [5, 5, 5, 5, 5]            # phase blocks
SUB = 7                             # groups per phase3 broadcast prefetch
EPS = 1e-5

AF = mybir.ActivationFunctionType
ALU = mybir.AluOpType
dt = mybir.dt


# --------------------------------------------------------------------------
# device kernel builder
# --------------------------------------------------------------------------

@functools.lru_cache(maxsize=4)
def _build(cis: tuple, n_cores: int, affine_id: bool):
    assert len(cis) == TILES_PER_CORE
    coff = np.concatenate([[0], np.cumsum(cis)]).astype(int)
    # group chunk counts and pk byte offsets (384 B per chunk-column)
    gch = [int(coff[4 * g + 4] - coff[4 * g]) for g in range(N_GROUPS)]
    gbytes = [c * 384 for c in gch]
    boff = np.concatenate([[0], np.cumsum(gbytes)]).astype(int)
    gbmax = max(gbytes)

    blocks = []
    g0 = 0
    for n in BLOCKS:
        blocks.append(list(range(g0, g0 + n)))
        g0 += n
    assert g0 == N_GROUPS
    bmax = max(len(b) for b in blocks)

    nc = bacc.Bacc("TRN2", target_bir_lowering=False, debug=False,
                   enable_asserts=False, num_devices=n_cores)

    PK = nc.dram_tensor("pk", [P, int(boff[-1])], dt.uint8,
                        kind="ExternalInput").ap()
    NTB = nc.dram_tensor("ntb", [P, NODES_PER_CORE], dt.bfloat16,
                         kind="ExternalInput").ap()
    W1P = nc.dram_tensor("w1p", [P, 1024], dt.bfloat16, kind="ExternalInput").ap()
    W2P = nc.dram_tensor("w2p", [P, 512], dt.bfloat16, kind="ExternalInput").ap()
    B1P = nc.dram_tensor("b1p", [P, 4], dt.float32, kind="ExternalInput").ap()
    B2P = nc.dram_tensor("b2p", [P, 1], dt.float32, kind="ExternalInput").ap()
    GAM = nc.dram_tensor("gam", [P, 1], dt.float32, kind="ExternalInput").ap()
    BET = nc.dram_tensor("bet", [P, 1], dt.float32, kind="ExternalInput").ap()
    ONB = nc.dram_tensor("onb", [P, bmax * 128], dt.bfloat16,
                         kind="ExternalInput").ap()
    SEL = nc.dram_tensor("sel", [P, bmax * 128], dt.bfloat16,
                         kind="ExternalInput").ap()
    OUT = nc.dram_tensor("out", [P, NODES_PER_CORE], dt.bfloat16,
                         kind="ExternalOutput").ap()

    with tile.TileContext(nc) as tc:
        with ExitStack() as ctx:
            singles = ctx.enter_context(tc.tile_pool(name="singles", bufs=1))
            pkp = ctx.enter_context(tc.tile_pool(name="pkp", bufs=5))
            xtp = ctx.enter_context(tc.tile_pool(name="xtp", bufs=18))
            xap = ctx.enter_context(tc.tile_pool(name="xap", bufs=3))
            shp = ctx.enter_context(tc.tile_pool(name="shp", bufs=6))
            yp = ctx.enter_context(tc.tile_pool(name="yp", bufs=18))
            y2p = ctx.enter_context(tc.tile_pool(name="y2p", bufs=bmax + 2))
            stp = ctx.enter_context(tc.tile_pool(name="stp", bufs=2))
            zp = ctx.enter_context(tc.tile_pool(name="zp", bufs=3))
            psagg = ctx.enter_context(tc.tile_pool(name="psagg", bufs=1, space="PSUM"))
            psh = ctx.enter_context(tc.tile_pool(name="psh", bufs=2, space="PSUM"))
            psy = ctx.enter_context(tc.tile_pool(name="psy", bufs=1, space="PSUM"))
            psst = ctx.enter_context(tc.tile_pool(name="psst", bufs=1, space="PSUM"))
            psbc = ctx.enter_context(tc.tile_pool(name="psbc", bufs=1, space="PSUM"))

            def load_const(name, src, shape, dtyp):
                t = singles.tile(shape, dtyp, tag=name)
                nc.sync.dma_start(out=t[:], in_=src)
                return t

            # prime the edge-stream pipeline before the (later-needed) consts
            pre_pk = pkp.tile([P, gbmax], dt.uint8, tag="pk")
            nc.sync.dma_start(out=pre_pk[:, :gbytes[0]],
                              in_=PK[:, 0:gbytes[0]])

            w1 = load_const("w1", W1P, [P, 1024], dt.bfloat16)
            w2 = load_const("w2", W2P, [P, 512], dt.bfloat16)
            b1 = load_const("b1", B1P, [P, 4], dt.float32)
            b2 = load_const("b2", B2P, [P, 1], dt.float32)
            gam = load_const("gam", GAM, [P, 1], dt.float32)
            bet = load_const("bet", BET, [P, 1], dt.float32)
            onb = load_const("onb", ONB, [P, bmax * 128], dt.bfloat16)
            sel = load_const("sel", SEL, [P, bmax * 128], dt.bfloat16)
            magic = singles.tile([P, GROUP], dt.uint16, tag="magic")
            nc.vector.memset(magic[:], 0x5f37)

            pk_tiles = {0: pre_pk}
            xtn_tiles = {}
            y_tiles = {}
            y2_tiles = {}
            out_stage = [None]

            def dma_group(g):
                if g >= N_GROUPS:
                    return
                if g not in pk_tiles:
                    pkt = pkp.tile([P, gbmax], dt.uint8, tag="pk")
                    nc.sync.dma_start(
                        out=pkt[:, :gbytes[g]],
                        in_=PK[:, int(boff[g]):int(boff[g]) + gbytes[g]])
                    pk_tiles[g] = pkt
                # node features load 2 groups per DMA (fewer 1KB descriptors)
                if g not in xtn_tiles:
                    n2 = 2 if g + 1 < N_GROUPS else 1
                    xt2 = xtp.tile([P, 1024], dt.bfloat16, tag="xtn")
                    nc.sync.dma_start(
                        out=xt2[:, :n2 * GROUP],
                        in_=NTB[:, g * GROUP:(g + n2) * GROUP])
                    xtn_tiles[g] = xt2[:, 0:GROUP]
                    if n2 == 2:
                        xtn_tiles[g + 1] = xt2[:, GROUP:2 * GROUP]

            def scatter_mlp(g):
                agg_ps = psagg.tile([P, GROUP], dt.float32, tag="agg")
                pkt = pk_tiles.pop(g)
                a = int(coff[4 * g])
                for t4 in range(4):
                    ti = 4 * g + t4
                    ci = int(cis[ti])
                    toff = (int(coff[ti]) - a) * 384
                    ebv = pkt[:, toff:toff + ci * 256].bitcast(dt.bfloat16)
                    ohv = pkt[:, toff + ci * 256:toff + ci * 384].bitcast(
                        dt.float8e4)
                    for c in range(ci):
                        nc.tensor.matmul(
                            out=agg_ps[:, t4 * 128:(t4 + 1) * 128],
                            lhsT=ebv[:, c * 128:(c + 1) * 128],
                            rhs=ohv[:, c * 128:(c + 1) * 128],
                            start=(c == 0), stop=(c == ci - 1))
                xta = xap.tile([P, GROUP], dt.bfloat16, tag="xta")
                nc.scalar.activation(out=xta[:], in_=agg_ps[:], func=AF.Copy)
                xtn = xtn_tiles[g]
                sh_tiles = []
                for j in range(4):
                    hps = psh.tile([P, GROUP], dt.float32, tag="hps")
                    nc.tensor.matmul(out=hps[:],
                                     lhsT=w1[:, j * 128:(j + 1) * 128],
                                     rhs=xtn[:], start=True, stop=False)
                    nc.tensor.matmul(
                        out=hps[:],
                        lhsT=w1[:, 512 + j * 128:512 + (j + 1) * 128],
                        rhs=xta[:], start=False, stop=True)
                    sh = shp.tile([P, GROUP], dt.bfloat16, tag=f"sh{j}")
                    nc.scalar.activation(out=sh[:], in_=hps[:], func=AF.Silu,
                                         bias=b1[:, j:j + 1], scale=1.0)
                    sh_tiles.append(sh)
                yps = psy.tile([P, GROUP], dt.float32, tag="yps")
                for j in range(4):
                    nc.tensor.matmul(out=yps[:],
                                     lhsT=w2[:, j * 128:(j + 1) * 128],
                                     rhs=sh_tiles[j][:],
                                     start=(j == 0), stop=(j == 3))
                y = yp.tile([P, GROUP], dt.bfloat16, tag="y")
                nc.vector.tensor_scalar(out=y[:], in0=yps[:],
                                        scalar1=b2[:, 0:1], scalar2=None,
                                        op0=ALU.add)
                y_tiles[g] = y
                y2 = y2p.tile([P, GROUP], dt.bfloat16, tag="y2")
                nc.vector.tensor_tensor(out=y2[:], in0=y[:], in1=y[:],
                                        op=ALU.mult)
                y2_tiles[g] = y2

            def stats_burst(block):
                bsz = len(block)
                mu_ps = psst.tile([P, GROUP], dt.float32, tag="mups")
                m2_ps = psst.tile([P, GROUP], dt.float32, tag="m2ps")
                for gi, g in enumerate(block):
                    onc_g = onb[:, gi * 128:(gi + 1) * 128]
                    nc.tensor.matmul(out=mu_ps[:], lhsT=onc_g,
                                     rhs=y_tiles[g][:],
                                     start=(gi == 0), stop=(gi == bsz - 1),
                                     skip_group_check=True)
                    nc.tensor.matmul(out=m2_ps[:], lhsT=onc_g,
                                     rhs=y2_tiles.pop(g)[:],
                                     start=(gi == 0), stop=(gi == bsz - 1),
                                     skip_group_check=True)
                return mu_ps, m2_ps

            def phase2(block, mu_ps, m2_ps):
                bsz = len(block)
                mu_bf = stp.tile([P, GROUP], dt.bfloat16, tag="mubf")
                nc.scalar.activation(out=mu_bf[:], in_=mu_ps[:], func=AF.Copy)
                musq = stp.tile([P, GROUP], dt.bfloat16, tag="musq")
                nc.scalar.activation(out=musq[:], in_=mu_ps[:], func=AF.Square)
                m2_bf = stp.tile([P, GROUP], dt.bfloat16, tag="m2bf")
                nc.scalar.activation(out=m2_bf[:], in_=m2_ps[:], func=AF.Copy)
                # var + eps = (m2 + eps) - mu^2 (TT/TS run 2x/4x; STT is 1x)
                m2e = stp.tile([P, GROUP], dt.bfloat16, tag="m2e")
                nc.vector.tensor_scalar(out=m2e[:], in0=m2_bf[:],
                                        scalar1=EPS, scalar2=None,
                                        op0=ALU.add)
                var = stp.tile([P, GROUP], dt.bfloat16, tag="var")
                nc.vector.tensor_tensor(out=var[:], in0=m2e[:], in1=musq[:],
                                        op=ALU.subtract)
                # rstd = rsqrt(var) via bf16 bit-hack + 2 Newton steps, all
                # on the vector engine -- avoids the Ln/Exp ACT table set
                # (each set swap stalls silu for ~2.7us at block boundaries)
                sh = stp.tile([P, GROUP], dt.uint16, tag="sh")
                nc.vector.tensor_scalar(out=sh[:], in0=var[:].bitcast(dt.uint16),
                                        scalar1=1, scalar2=None,
                                        op0=ALU.logical_shift_right)
                r = stp.tile([P, GROUP], dt.uint16, tag="r0")
                nc.vector.tensor_tensor(out=r[:], in0=magic[:], in1=sh[:],
                                        op=ALU.subtract)
                r = r[:].bitcast(dt.bfloat16)
                # [D | rstd] packed in one tile so one selector matmul
                # broadcasts both; final Newton step lands in the rstd half
                dr = stp.tile([P, 1024], dt.bfloat16, tag="dr")
                for it in range(2):
                    t = stp.tile([P, GROUP], dt.bfloat16, tag=f"nt{it}")
                    nc.vector.tensor_tensor(out=t[:], in0=r, in1=r,
                                            op=ALU.mult)
                    nc.vector.tensor_tensor(out=t[:], in0=t[:], in1=var[:],
                                            op=ALU.mult)
                    nc.vector.tensor_scalar(out=t[:], in0=t[:],
                                            scalar1=-0.5, scalar2=1.5,
                                            op0=ALU.mult, op1=ALU.add)
                    if it == 1:
                        rn = dr[:, 512:1024]
                    else:
                        rtile = stp.tile([P, GROUP], dt.bfloat16,
                                         tag=f"nr{it}")
                        rn = rtile[:]
                    nc.vector.tensor_tensor(out=rn, in0=r, in1=t[:],
                                            op=ALU.mult)
                    r = rn
                nc.vector.tensor_tensor(out=dr[:, 0:512], in0=mu_bf[:],
                                        in1=dr[:, 512:1024], op=ALU.mult)
                return dr

            def phase3(g, gi, block, dr):
                # broadcast row gi of [D | rstd] across partitions with a
                # row-selector matmul (keeps the 128x-replicated reads off
                # the saturated DMA queues)
                sel_g = sel[:, gi * 128:(gi + 1) * 128]
                mr_ps = psbc.tile([P, 1024], dt.float32, tag="mrps")
                nc.tensor.matmul(out=mr_ps[:, 0:512], lhsT=sel_g,
                                 rhs=dr[:, 0:512], start=True, stop=True)
                nc.tensor.matmul(out=mr_ps[:, 512:1024], lhsT=sel_g,
                                 rhs=dr[:, 512:1024], start=True, stop=True)
                mr_d = mr_ps[:, 0:512]
                mr_r = mr_ps[:, 512:1024]
                nsl = slice(g * GROUP, (g + 1) * GROUP)
                y = y_tiles.pop(g)
                xtn = xtn_tiles.pop(g)
                t1 = zp.tile([P, GROUP], dt.bfloat16, tag="t1")
                nc.vector.tensor_tensor(out=t1[:], in0=y[:], in1=mr_r,
                                        op=ALU.mult)
                t2 = zp.tile([P, GROUP], dt.bfloat16, tag="t2")
                nc.vector.tensor_tensor(out=t2[:], in0=t1[:], in1=mr_d,
                                        op=ALU.subtract)
                # stage two consecutive groups' outputs in one [P,1024] tile
                # so each OUT DMA moves 2KB rows instead of 1KB
                if g % 2 == 0:
                    op2 = zp.tile([P, 1024], dt.bfloat16, tag="op2")
                    out_stage[0] = op2
                    of = op2[:, 0:GROUP]
                else:
                    op2 = out_stage[0]
                    of = op2[:, GROUP:2 * GROUP]
                if affine_id:
                    nc.vector.tensor_tensor(out=of, in0=t2[:], in1=xtn[:],
                                            op=ALU.add)
                else:
                    t3 = zp.tile([P, GROUP], dt.bfloat16, tag="t3")
                    nc.vector.tensor_scalar(out=t3[:], in0=t2[:],
                                            scalar1=gam[:, 0:1],
                                            scalar2=bet[:, 0:1],
                                            op0=ALU.mult, op1=ALU.add)
                    nc.vector.tensor_tensor(out=of, in0=t3[:], in1=xtn[:],
                                            op=ALU.add)
                if g % 2 == 1:
                    nc.gpsimd.dma_start(
                        out=OUT[:, (g - 1) * GROUP:(g + 1) * GROUP],
                        in_=op2[:])
                elif g == N_GROUPS - 1:
                    nc.gpsimd.dma_start(out=OUT[:, nsl], in_=op2[:, 0:GROUP])

            # ---- emission ----
            LOOK = 3
            for g0_ in range(LOOK):
                dma_group(g0_)
            pending = []   # (g, gi, block, dr) phase3 work not yet emitted
            for bi, block in enumerate(blocks):
                for g in block:
                    dma_group(g + LOOK)
                    scatter_mlp(g)
                    # interleave pending normalize work 1:1 with phase1
                    if pending:
                        phase3(*pending.pop(0))
                mu_ps, m2_ps = stats_burst(block)
                dr = phase2(block, mu_ps, m2_ps)
                pending.extend(
                    (g2, gi, block, dr) for gi, g2 in enumerate(block))
            # leftovers run after all PE phase1 work (PE is done anyway)
            for item in pending:
                phase3(*item)

    nc.compile()
    return nc


# --------------------------------------------------------------------------
# host-side sharding / packing
# --------------------------------------------------------------------------

def _preprocess(inputs):
    nf = np.ascontiguousarray(np.asarray(inputs["node_features"], np.float32))
    ef = np.ascontiguousarray(np.asarray(inputs["edge_features"], np.float32))
    src = np.asarray(inputs["src_indices"]).astype(np.int64)
    W1 = np.asarray(inputs["W1"], np.float32)
    b1 = np.asarray(inputs["b1"], np.float32)
    W2 = np.asarray(inputs["W2"], np.float32)
    b2 = np.asarray(inputs["b2"], np.float32)
    gam = np.asarray(inputs["ln_gamma"], np.float32)
    bet = np.asarray(inputs["ln_beta"], np.float32)

    n_nodes, d = nf.shape
    n_edges = ef.shape[0]
    assert n_nodes == N_NODES and d == D

    # degree-balanced snake deal of nodes into 800 bins of 128 slots
    deg = np.bincount(src, minlength=n_nodes)
    order = np.argsort(-deg, kind="stable")
    idx = np.arange(n_nodes)
    r = idx // NBINS
    c = idx % NBINS
    b = np.where(r % 2 == 0, c, NBINS - 1 - c)
    bin_of = np.empty(n_nodes, np.int64)
    slot_of = np.empty(n_nodes, np.int64)
    bin_of[order] = b
    slot_of[order] = r
    assert slot_of.max() < P

    bindeg = np.bincount(bin_of, weights=deg, minlength=NBINS).astype(np.int64)
    cis = np.ceil(bindeg.reshape(N_CORES, TILES_PER_CORE) / P).astype(int)
    cis = np.maximum(cis.max(axis=0), 1)
    coff = np.concatenate([[0], np.cumsum(cis)]).astype(int)
    chtot = int(coff[-1])

    # edge placement: sort by (bin, lid), chunk within bin
    eb = bin_of[src]
    lid = slot_of[src]
    eorder = np.argsort(eb * P + lid, kind="stable")
    sb = eb[eorder]
    counts = np.bincount(eb, minlength=NBINS)
    starts = np.concatenate([[0], np.cumsum(counts)[:-1]])
    rank = np.arange(n_edges, dtype=np.int64) - starts[sb]
    chunk = rank // P
    pslot = rank % P
    core_e = sb // TILES_PER_CORE
    ti_e = sb % TILES_PER_CORE
    gc = coff[ti_e] + chunk

    EB = np.zeros((N_CORES, P, chtot, D), BF16)
    EB[core_e, pslot, gc, :] = ef[eorder].astype(BF16)
    OH = np.zeros((N_CORES, P, chtot, P), FP8)
    OH[core_e, pslot, gc, lid[eorder]] = 1.0

    EB8 = EB.view(np.uint8).reshape(N_CORES, P, chtot * 256)
    OH8 = OH.view(np.uint8).reshape(N_CORES, P, chtot * 128)
    parts = []
    for ti in range(TILES_PER_CORE):
        a, e = int(coff[ti]), int(coff[ti + 1])
        parts.append(EB8[:, :, a * 256:e * 256])
        parts.append(OH8[:, :, a * 128:e * 128])
    PKa = np.ascontiguousarray(np.concatenate(parts, axis=2))

    # permuted node features, transposed
    core_n = bin_of // TILES_PER_CORE
    col_n = (bin_of % TILES_PER_CORE) * P + slot_of
    NT = np.zeros((N_CORES, NODES_PER_CORE, D), np.float32)
    NT[core_n, col_n] = nf
    NTBa = np.ascontiguousarray(NT.transpose(0, 2, 1)).astype(BF16)

    W1P = np.ascontiguousarray(
        W1.reshape(2, P, 4, P).transpose(1, 0, 2, 3).reshape(P, 1024)).astype(BF16)
    W2P = np.ascontiguousarray(
        W2.reshape(4, P, P).transpose(1, 0, 2).reshape(P, 512)).astype(BF16)
    B1P = np.ascontiguousarray(b1.reshape(4, P).T)
    B2P = np.ascontiguousarray(b2.reshape(P, 1))
    GAMP = np.ascontiguousarray(gam.reshape(P, 1))
    BETP = np.ascontiguousarray(bet.reshape(P, 1))
    bmax = max(BLOCKS)
    ONBa = np.zeros((P, bmax * 128), np.float32)
    for g in range(bmax):
        ONBa[:, g * 128 + g] = 1.0 / P
    ONBa = ONBa.astype(BF16)
    SELa = np.zeros((P, bmax * 128), np.float32)
    for g in range(bmax):
        SELa[g, g * 128:(g + 1) * 128] = 1.0
    SELa = SELa.astype(BF16)

    in_maps = []
    for k in range(N_CORES):
        in_maps.append({
            "pk": PKa[k], "ntb": NTBa[k],
            "w1p": W1P, "w2p": W2P, "b1p": B1P, "b2p": B2P,
            "gam": GAMP, "bet": BETP, "onb": ONBa, "sel": SELa,
        })
    meta = (core_n, col_n)
    affine_id = bool(np.all(bet == 0.0) and np.all(gam == 1.0))
    return in_maps, tuple(int(x) for x in cis), affine_id, meta


def _assemble(results, meta):
    core_n, col_n = meta
    outs = np.stack([np.asarray(r["out"]) for r in results]).astype(np.float32)
    full = outs[core_n, :, col_n]          # [n_nodes, D]
    return np.ascontiguousarray(full)


# --------------------------------------------------------------------------
# public entry point
# --------------------------------------------------------------------------

_AXON_SO = "/opt/axon/libaxon_pjrt.so"


def _ensure_ntff_hook():
    """Provide antenv.axon_hooks + register the ctypes NTFF profile hook
    (the agent image's antenv lacks axon_hooks, so boot degraded silently)."""
    import sys
    import types
    import ctypes
    import contextlib
    import os

    try:
        from antenv.axon_hooks import get_axon_ntff_profile_hook  # noqa: F401
        return
    except ImportError:
        pass
    import antenv

    m = types.ModuleType("antenv.axon_hooks")
    m._hook = None

    def set_axon_ntff_profile_hook(h):
        m._hook = h

    def get_axon_ntff_profile_hook():
        return m._hook

    m.set_axon_ntff_profile_hook = set_axon_ntff_profile_hook
    m.get_axon_ntff_profile_hook = get_axon_ntff_profile_hook
    sys.modules["antenv.axon_hooks"] = m
    antenv.axon_hooks = m

    if not os.path.exists(_AXON_SO):
        return
    lib = ctypes.CDLL(_AXON_SO)
    if not hasattr(lib, "axon_start_nrt_profile"):
        return
    lib.axon_start_nrt_profile.argtypes = [ctypes.POINTER(ctypes.c_int64),
                                           ctypes.c_size_t]
    lib.axon_start_nrt_profile.restype = ctypes.c_int64
    lib.axon_stop_nrt_profile.argtypes = [ctypes.c_char_p]
    lib.axon_stop_nrt_profile.restype = ctypes.c_int64

    @contextlib.contextmanager
    def _hook(output_dir, device_ids):
        import jax

        jax.devices()
        if device_ids:
            ids = (ctypes.c_int64 * len(device_ids))(*device_ids)
            rc = lib.axon_start_nrt_profile(ids, len(device_ids))
        else:
            rc = lib.axon_start_nrt_profile(None, 0)
        if rc != 0:
            raise RuntimeError(f"axon_start_nrt_profile rc={rc}")
        try:
            yield
        finally:
            n = lib.axon_stop_nrt_profile(str(output_dir).encode())
            if n < 0:
                raise RuntimeError(f"axon_stop_nrt_profile rc={n}")
            if n == 0:
                print("WARNING: NTFF capture wrote no files")

    m._hook = _hook


def _run(inputs, trace=False):
    if trace:
        _ensure_ntff_hook()
    in_maps, cis, affine_id, meta = _preprocess(inputs)
    nc = _build(cis, N_CORES, affine_id)
    res = bass_utils.run_bass_kernel_spmd(
        nc, in_maps, core_ids=list(range(N_CORES)), trace=trace)
    out = _assemble(res.results, meta)
    return out, res


def kernel(**inputs):
    out, _ = _run(inputs, trace=False)
    return out


def kernel_profiled(**inputs):
    out, res = _run(inputs, trace=True)
    return out, res


# revision 45
# speedup vs baseline: 1.1118x; 1.1118x over previous
"""Trainium2 Bass kernel for nn_MeshNodeBlock (GNN message passing block).

reference semantics:
    agg = segment_sum(edge_features, src_indices, N)        # scatter-add
    x   = concat([node_features, agg], -1)
    h   = silu(x @ W1 + b1)
    y   = h @ W2 + b2
    y   = layer_norm(y) * gamma + beta
    out = y + node_features

Strategy (8 NeuronCores, SPMD, one NEFF):
  * Host snake-deals nodes by degree into 800 bins (8 cores x 100 tiles) of
    128 slots each, so every tile receives ~750 edges = exactly 6 chunks of
    128 (a contiguous partition needs 7). Each chunk ships bf16 edge
    features (256 B/slot) + fp8 one-hot (128 B/slot).
  * Device works fully in transposed space (features on partitions, nodes on
    free dim). Per 128-node tile the scatter-add is ci PE matmuls
    aggT += edge_chunk.T @ onehot into the group's [128,512] PSUM tile.
  * MLP consumes aggT/nodeT directly: layer 1 -> silu(+b1) on the scalar
    engine, layer 2 -> yT. xta copy on scalar engine; y (+b2) and y^2 on
    the vector engine (y^2 from SBUF, 2x mode).
  * LayerNorm stats via ONCB matmuls (rows of a shared PSUM bank); block
    phase2 computes rstd (ln/exp set) and D = mu*rstd, writes [D|rstd] rows
    to a DRAM bounce; phase3 prefetches them partition-broadcast in one
    large DMA per 7-group sub-block (not 128 tiny replicated reads per
    group) and normalizes in 3-4 fused DVE ops
    (y*rstd - D)*gamma + beta + node, interleaved with next block's work.
  * Output written transposed in bf16; host inverts the node permutation.
"""

import functools
from contextlib import ExitStack

import numpy as np
import ml_dtypes

import concourse.bass as bass
import concourse.tile as tile
from concourse import bacc, mybir
from concourse import bass_utils

BF16 = ml_dtypes.bfloat16
FP8 = ml_dtypes.float8_e4m3

N_NODES = 100000
D = 128
N_CORES = 8
P = 128
GROUP = 512
N_GROUPS = 25
W = 64                              # scatter window (nodes per one-hot)
WPG = GROUP // W                    # windows per group = 8
TILES_PER_CORE = NODES_PER_CORE_T = 200
NODES_PER_CORE = N_GROUPS * GROUP   # 12800
NBINS = N_CORES * TILES_PER_CORE    # 1600
BLOCKS = [5, 5, 5, 5, 5]            # phase blocks
SUB = 7                             # groups per phase3 broadcast prefetch
EPS = 1e-5

AF = mybir.ActivationFunctionType
ALU = mybir.AluOpType
dt = mybir.dt


# --------------------------------------------------------------------------
# device kernel builder
# --------------------------------------------------------------------------

@functools.lru_cache(maxsize=4)
def _build(cis: tuple, n_cores: int, affine_id: bool):
    assert len(cis) == TILES_PER_CORE
    coff = np.concatenate([[0], np.cumsum(cis)]).astype(int)
    # group chunk counts and pk byte offsets (256 B edges + 64 B one-hot)
    gch = [int(coff[WPG * g + WPG] - coff[WPG * g]) for g in range(N_GROUPS)]
    gbytes = [c * 320 for c in gch]
    boff = np.concatenate([[0], np.cumsum(gbytes)]).astype(int)
    gbmax = max(gbytes)

    blocks = []
    g0 = 0
    for n in BLOCKS:
        blocks.append(list(range(g0, g0 + n)))
        g0 += n
    assert g0 == N_GROUPS
    bmax = max(len(b) for b in blocks)

    nc = bacc.Bacc("TRN2", target_bir_lowering=False, debug=False,
                   enable_asserts=False, num_devices=n_cores)

    PK = nc.dram_tensor("pk", [P, int(boff[-1])], dt.uint8,
                        kind="ExternalInput").ap()
    NTB = nc.dram_tensor("ntb", [P, NODES_PER_CORE], dt.bfloat16,
                         kind="ExternalInput").ap()
    W1P = nc.dram_tensor("w1p", [P, 1024], dt.bfloat16, kind="ExternalInput").ap()
    W2P = nc.dram_tensor("w2p", [P, 512], dt.bfloat16, kind="ExternalInput").ap()
    B1P = nc.dram_tensor("b1p", [P, 4], dt.float32, kind="ExternalInput").ap()
    B2P = nc.dram_tensor("b2p", [P, 1], dt.float32, kind="ExternalInput").ap()
    GAM = nc.dram_tensor("gam", [P, 1], dt.float32, kind="ExternalInput").ap()
    BET = nc.dram_tensor("bet", [P, 1], dt.float32, kind="ExternalInput").ap()
    ONB = nc.dram_tensor("onb", [P, bmax * 128], dt.bfloat16,
                         kind="ExternalInput").ap()
    SEL = nc.dram_tensor("sel", [P, bmax * 128], dt.bfloat16,
                         kind="ExternalInput").ap()
    OUT = nc.dram_tensor("out", [P, NODES_PER_CORE], dt.bfloat16,
                         kind="ExternalOutput").ap()

    with tile.TileContext(nc) as tc:
        with ExitStack() as ctx:
            singles = ctx.enter_context(tc.tile_pool(name="singles", bufs=1))
            pkp = ctx.enter_context(tc.tile_pool(name="pkp", bufs=5))
            xtp = ctx.enter_context(tc.tile_pool(name="xtp", bufs=18))
            xap = ctx.enter_context(tc.tile_pool(name="xap", bufs=3))
            shp = ctx.enter_context(tc.tile_pool(name="shp", bufs=6))
            yp = ctx.enter_context(tc.tile_pool(name="yp", bufs=18))
            y2p = ctx.enter_context(tc.tile_pool(name="y2p", bufs=bmax + 2))
            stp = ctx.enter_context(tc.tile_pool(name="stp", bufs=2))
            zp = ctx.enter_context(tc.tile_pool(name="zp", bufs=3))
            psagg = ctx.enter_context(tc.tile_pool(name="psagg", bufs=1, space="PSUM"))
            psh = ctx.enter_context(tc.tile_pool(name="psh", bufs=2, space="PSUM"))
            psy = ctx.enter_context(tc.tile_pool(name="psy", bufs=1, space="PSUM"))
            psst = ctx.enter_context(tc.tile_pool(name="psst", bufs=1, space="PSUM"))
            psbc = ctx.enter_context(tc.tile_pool(name="psbc", bufs=1, space="PSUM"))

            def load_const(name, src, shape, dtyp):
                t = singles.tile(shape, dtyp, tag=name)
                nc.sync.dma_start(out=t[:], in_=src)
                return t

            # prime the edge-stream pipeline before the (later-needed) consts
            pre_pk = pkp.tile([P, gbmax], dt.uint8, tag="pk")
            nc.sync.dma_start(out=pre_pk[:, :gbytes[0]],
                              in_=PK[:, 0:gbytes[0]])

            w1 = load_const("w1", W1P, [P, 1024], dt.bfloat16)
            w2 = load_const("w2", W2P, [P, 512], dt.bfloat16)
            b1 = load_const("b1", B1P, [P, 4], dt.float32)
            b2 = load_const("b2", B2P, [P, 1], dt.float32)
            gam = load_const("gam", GAM, [P, 1], dt.float32)
            bet = load_const("bet", BET, [P, 1], dt.float32)
            onb = load_const("onb", ONB, [P, bmax * 128], dt.bfloat16)
            sel = load_const("sel", SEL, [P, bmax * 128], dt.bfloat16)
            magic = singles.tile([P, GROUP], dt.uint16, tag="magic")
            nc.vector.memset(magic[:], 0x5f37)

            pk_tiles = {0: pre_pk}
            xtn_tiles = {}
            y_tiles = {}
            y2_tiles = {}
            out_stage = [None]

            def dma_group(g):
                if g >= N_GROUPS:
                    return
                if g not in pk_tiles:
                    pkt = pkp.tile([P, gbmax], dt.uint8, tag="pk")
                    nc.sync.dma_start(
                        out=pkt[:, :gbytes[g]],
                        in_=PK[:, int(boff[g]):int(boff[g]) + gbytes[g]])
                    pk_tiles[g] = pkt
                # node features load 2 groups per DMA (fewer 1KB descriptors)
                if g not in xtn_tiles:
                    n2 = 2 if g + 1 < N_GROUPS else 1
                    xt2 = xtp.tile([P, 1024], dt.bfloat16, tag="xtn")
                    nc.sync.dma_start(
                        out=xt2[:, :n2 * GROUP],
                        in_=NTB[:, g * GROUP:(g + n2) * GROUP])
                    xtn_tiles[g] = xt2[:, 0:GROUP]
                    if n2 == 2:
                        xtn_tiles[g + 1] = xt2[:, GROUP:2 * GROUP]

            def scatter_mlp(g):
                agg_ps = psagg.tile([P, GROUP], dt.float32, tag="agg")
                pkt = pk_tiles.pop(g)
                a = int(coff[WPG * g])
                for t4 in range(WPG):
                    ti = WPG * g + t4
                    ci = int(cis[ti])
                    toff = (int(coff[ti]) - a) * 320
                    ebv = pkt[:, toff:toff + ci * 256].bitcast(dt.bfloat16)
                    ohv = pkt[:, toff + ci * 256:toff + ci * 320].bitcast(
                        dt.float8e4)
                    for c in range(ci):
                        nc.tensor.matmul(
                            out=agg_ps[:, t4 * W:(t4 + 1) * W],
                            lhsT=ebv[:, c * 128:(c + 1) * 128],
                            rhs=ohv[:, c * W:(c + 1) * W],
                            start=(c == 0), stop=(c == ci - 1))
                xta = xap.tile([P, GROUP], dt.bfloat16, tag="xta")
                nc.scalar.activation(out=xta[:], in_=agg_ps[:], func=AF.Copy)
                xtn = xtn_tiles[g]
                sh_tiles = []
                for j in range(4):
                    hps = psh.tile([P, GROUP], dt.float32, tag="hps")
                    nc.tensor.matmul(out=hps[:],
                                     lhsT=w1[:, j * 128:(j + 1) * 128],
                                     rhs=xtn[:], start=True, stop=False)
                    nc.tensor.matmul(
                        out=hps[:],
                        lhsT=w1[:, 512 + j * 128:512 + (j + 1) * 128],
                        rhs=xta[:], start=False, stop=True)
                    sh = shp.tile([P, GROUP], dt.bfloat16, tag=f"sh{j}")
                    nc.scalar.activation(out=sh[:], in_=hps[:], func=AF.Silu,
                                         bias=b1[:, j:j + 1], scale=1.0)
                    sh_tiles.append(sh)
                yps = psy.tile([P, GROUP], dt.float32, tag="yps")
                for j in range(4):
                    nc.tensor.matmul(out=yps[:],
                                     lhsT=w2[:, j * 128:(j + 1) * 128],
                                     rhs=sh_tiles[j][:],
                                     start=(j == 0), stop=(j == 3))
                y = yp.tile([P, GROUP], dt.bfloat16, tag="y")
                nc.vector.tensor_scalar(out=y[:], in0=yps[:],
                                        scalar1=b2[:, 0:1], scalar2=None,
                                        op0=ALU.add)
                y_tiles[g] = y
                y2 = y2p.tile([P, GROUP], dt.bfloat16, tag="y2")
                nc.vector.tensor_tensor(out=y2[:], in0=y[:], in1=y[:],
                                        op=ALU.mult)
                y2_tiles[g] = y2

            def stats_burst(block):
                bsz = len(block)
                mu_ps = psst.tile([P, GROUP], dt.float32, tag="mups")
                m2_ps = psst.tile([P, GROUP], dt.float32, tag="m2ps")
                for gi, g in enumerate(block):
                    onc_g = onb[:, gi * 128:(gi + 1) * 128]
                    nc.tensor.matmul(out=mu_ps[:], lhsT=onc_g,
                                     rhs=y_tiles[g][:],
                                     start=(gi == 0), stop=(gi == bsz - 1),
                                     skip_group_check=True)
                    nc.tensor.matmul(out=m2_ps[:], lhsT=onc_g,
                                     rhs=y2_tiles.pop(g)[:],
                                     start=(gi == 0), stop=(gi == bsz - 1),
                                     skip_group_check=True)
                return mu_ps, m2_ps

            def phase2(block, mu_ps, m2_ps):
                bsz = len(block)
                mu_bf = stp.tile([P, GROUP], dt.bfloat16, tag="mubf")
                nc.scalar.activation(out=mu_bf[:], in_=mu_ps[:], func=AF.Copy)
                musq = stp.tile([P, GROUP], dt.bfloat16, tag="musq")
                nc.scalar.activation(out=musq[:], in_=mu_ps[:], func=AF.Square)
                m2_bf = stp.tile([P, GROUP], dt.bfloat16, tag="m2bf")
                nc.scalar.activation(out=m2_bf[:], in_=m2_ps[:], func=AF.Copy)
                # var + eps = (m2 + eps) - mu^2 (TT/TS run 2x/4x; STT is 1x)
                m2e = stp.tile([P, GROUP], dt.bfloat16, tag="m2e")
                nc.vector.tensor_scalar(out=m2e[:], in0=m2_bf[:],
                                        scalar1=EPS, scalar2=None,
                                        op0=ALU.add)
                var = stp.tile([P, GROUP], dt.bfloat16, tag="var")
                nc.vector.tensor_tensor(out=var[:], in0=m2e[:], in1=musq[:],
                                        op=ALU.subtract)
                # rstd = rsqrt(var) via bf16 bit-hack + 2 Newton steps, all
                # on the vector engine -- avoids the Ln/Exp ACT table set
                # (each set swap stalls silu for ~2.7us at block boundaries)
                sh = stp.tile([P, GROUP], dt.uint16, tag="sh")
                nc.vector.tensor_scalar(out=sh[:], in0=var[:].bitcast(dt.uint16),
                                        scalar1=1, scalar2=None,
                                        op0=ALU.logical_shift_right)
                r = stp.tile([P, GROUP], dt.uint16, tag="r0")
                nc.vector.tensor_tensor(out=r[:], in0=magic[:], in1=sh[:],
                                        op=ALU.subtract)
                r = r[:].bitcast(dt.bfloat16)
                # [D | rstd] packed in one tile so one selector matmul
                # broadcasts both; final Newton step lands in the rstd half
                dr = stp.tile([P, 1024], dt.bfloat16, tag="dr")
                for it in range(2):
                    t = stp.tile([P, GROUP], dt.bfloat16, tag=f"nt{it}")
                    nc.vector.tensor_tensor(out=t[:], in0=r, in1=r,
                                            op=ALU.mult)
                    nc.vector.tensor_tensor(out=t[:], in0=t[:], in1=var[:],
                                            op=ALU.mult)
                    nc.vector.tensor_scalar(out=t[:], in0=t[:],
                                            scalar1=-0.5, scalar2=1.5,
                                            op0=ALU.mult, op1=ALU.add)
                    if it == 1:
                        rn = dr[:, 512:1024]
                    else:
                        rtile = stp.tile([P, GROUP], dt.bfloat16,
                                         tag=f"nr{it}")
                        rn = rtile[:]
                    nc.vector.tensor_tensor(out=rn, in0=r, in1=t[:],
                                            op=ALU.mult)
                    r = rn
                nc.vector.tensor_tensor(out=dr[:, 0:512], in0=mu_bf[:],
                                        in1=dr[:, 512:1024], op=ALU.mult)
                return dr

            def phase3(g, gi, block, dr):
                # broadcast row gi of [D | rstd] across partitions with a
                # row-selector matmul (keeps the 128x-replicated reads off
                # the saturated DMA queues)
                sel_g = sel[:, gi * 128:(gi + 1) * 128]
                mr_ps = psbc.tile([P, 1024], dt.float32, tag="mrps")
                nc.tensor.matmul(out=mr_ps[:, 0:512], lhsT=sel_g,
                                 rhs=dr[:, 0:512], start=True, stop=True)
                nc.tensor.matmul(out=mr_ps[:, 512:1024], lhsT=sel_g,
                                 rhs=dr[:, 512:1024], start=True, stop=True)
                mr_d = mr_ps[:, 0:512]
                mr_r = mr_ps[:, 512:1024]
                nsl = slice(g * GROUP, (g + 1) * GROUP)
                y = y_tiles.pop(g)
                xtn = xtn_tiles.pop(g)
                t1 = zp.tile([P, GROUP], dt.bfloat16, tag="t1")
                nc.vector.tensor_tensor(out=t1[:], in0=y[:], in1=mr_r,
                                        op=ALU.mult)
                t2 = zp.tile([P, GROUP], dt.bfloat16, tag="t2")
                nc.vector.tensor_tensor(out=t2[:], in0=t1[:], in1=mr_d,
                                        op=ALU.subtract)
                # stage two consecutive groups' outputs in one [P,1024] tile
                # so each OUT DMA moves 2KB rows instead of 1KB
                if g % 2 == 0:
                    op2 = zp.tile([P, 1024], dt.bfloat16, tag="op2")
                    out_stage[0] = op2
                    of = op2[:, 0:GROUP]
                else:
                    op2 = out_stage[0]
                    of = op2[:, GROUP:2 * GROUP]
                if affine_id:
                    nc.vector.tensor_tensor(out=of, in0=t2[:], in1=xtn[:],
                                            op=ALU.add)
                else:
                    t3 = zp.tile([P, GROUP], dt.bfloat16, tag="t3")
                    nc.vector.tensor_scalar(out=t3[:], in0=t2[:],
                                            scalar1=gam[:, 0:1],
                                            scalar2=bet[:, 0:1],
                                            op0=ALU.mult, op1=ALU.add)
                    nc.vector.tensor_tensor(out=of, in0=t3[:], in1=xtn[:],
                                            op=ALU.add)
                if g % 2 == 1:
                    nc.gpsimd.dma_start(
                        out=OUT[:, (g - 1) * GROUP:(g + 1) * GROUP],
                        in_=op2[:])
                elif g == N_GROUPS - 1:
                    nc.gpsimd.dma_start(out=OUT[:, nsl], in_=op2[:, 0:GROUP])

            # ---- emission ----
            LOOK = 3
            for g0_ in range(LOOK):
                dma_group(g0_)
            pending = []   # (g, gi, block, dr) phase3 work not yet emitted
            for bi, block in enumerate(blocks):
                for g in block:
                    dma_group(g + LOOK)
                    scatter_mlp(g)
                    # interleave pending normalize work 1:1 with phase1
                    if pending:
                        phase3(*pending.pop(0))
                mu_ps, m2_ps = stats_burst(block)
                dr = phase2(block, mu_ps, m2_ps)
                pending.extend(
                    (g2, gi, block, dr) for gi, g2 in enumerate(block))
            # leftovers run after all PE phase1 work (PE is done anyway)
            for item in pending:
                phase3(*item)

    nc.compile()
    return nc


# --------------------------------------------------------------------------
# host-side sharding / packing
# --------------------------------------------------------------------------

def _preprocess(inputs):
    nf = np.ascontiguousarray(np.asarray(inputs["node_features"], np.float32))
    ef = np.ascontiguousarray(np.asarray(inputs["edge_features"], np.float32))
    src = np.asarray(inputs["src_indices"]).astype(np.int64)
    W1 = np.asarray(inputs["W1"], np.float32)
    b1 = np.asarray(inputs["b1"], np.float32)
    W2 = np.asarray(inputs["W2"], np.float32)
    b2 = np.asarray(inputs["b2"], np.float32)
    gam = np.asarray(inputs["ln_gamma"], np.float32)
    bet = np.asarray(inputs["ln_beta"], np.float32)

    n_nodes, d = nf.shape
    n_edges = ef.shape[0]
    assert n_nodes == N_NODES and d == D

    # degree-balanced snake deal of nodes into 800 bins of 128 slots
    deg = np.bincount(src, minlength=n_nodes)
    order = np.argsort(-deg, kind="stable")
    idx = np.arange(n_nodes)
    r = idx // NBINS
    c = idx % NBINS
    b = np.where(r % 2 == 0, c, NBINS - 1 - c)
    bin_of = np.empty(n_nodes, np.int64)
    slot_of = np.empty(n_nodes, np.int64)
    bin_of[order] = b
    slot_of[order] = r
    assert slot_of.max() < W

    bindeg = np.bincount(bin_of, weights=deg, minlength=NBINS).astype(np.int64)
    cis = np.ceil(bindeg.reshape(N_CORES, TILES_PER_CORE) / P).astype(int)
    cis = np.maximum(cis.max(axis=0), 1)
    coff = np.concatenate([[0], np.cumsum(cis)]).astype(int)
    chtot = int(coff[-1])

    # edge placement: sort by (bin, lid), chunk within bin
    eb = bin_of[src]
    lid = slot_of[src]
    eorder = np.argsort(eb * W + lid, kind="stable")
    sb = eb[eorder]
    counts = np.bincount(eb, minlength=NBINS)
    starts = np.concatenate([[0], np.cumsum(counts)[:-1]])
    rank = np.arange(n_edges, dtype=np.int64) - starts[sb]
    chunk = rank // P
    pslot = rank % P
    core_e = sb // TILES_PER_CORE
    ti_e = sb % TILES_PER_CORE
    gc = coff[ti_e] + chunk

    EB = np.zeros((N_CORES, P, chtot, D), BF16)
    EB[core_e, pslot, gc, :] = ef[eorder].astype(BF16)
    OH = np.zeros((N_CORES, P, chtot, W), FP8)
    OH[core_e, pslot, gc, lid[eorder]] = 1.0

    EB8 = EB.view(np.uint8).reshape(N_CORES, P, chtot * 256)
    OH8 = OH.view(np.uint8).reshape(N_CORES, P, chtot * W)
    parts = []
    for ti in range(TILES_PER_CORE):
        a, e = int(coff[ti]), int(coff[ti + 1])
        parts.append(EB8[:, :, a * 256:e * 256])
        parts.append(OH8[:, :, a * W:e * W])
    PKa = np.ascontiguousarray(np.concatenate(parts, axis=2))

    # permuted node features, transposed
    core_n = bin_of // TILES_PER_CORE
    col_n = (bin_of % TILES_PER_CORE) * W + slot_of
    NT = np.zeros((N_CORES, NODES_PER_CORE, D), np.float32)
    NT[core_n, col_n] = nf
    NTBa = np.ascontiguousarray(NT.transpose(0, 2, 1)).astype(BF16)

    W1P = np.ascontiguousarray(
        W1.reshape(2, P, 4, P).transpose(1, 0, 2, 3).reshape(P, 1024)).astype(BF16)
    W2P = np.ascontiguousarray(
        W2.reshape(4, P, P).transpose(1, 0, 2).reshape(P, 512)).astype(BF16)
    B1P = np.ascontiguousarray(b1.reshape(4, P).T)
    B2P = np.ascontiguousarray(b2.reshape(P, 1))
    GAMP = np.ascontiguousarray(gam.reshape(P, 1))
    BETP = np.ascontiguousarray(bet.reshape(P, 1))
    bmax = max(BLOCKS)
    ONBa = np.zeros((P, bmax * 128), np.float32)
    for g in range(bmax):
        ONBa[:, g * 128 + g] = 1.0 / P
    ONBa = ONBa.astype(BF16)
    SELa = np.zeros((P, bmax * 128), np.float32)
    for g in range(bmax):
        SELa[g, g * 128:(g + 1) * 128] = 1.0
    SELa = SELa.astype(BF16)

    in_maps = []
    for k in range(N_CORES):
        in_maps.append({
            "pk": PKa[k], "ntb": NTBa[k],
            "w1p": W1P, "w2p": W2P, "b1p": B1P, "b2p": B2P,
            "gam": GAMP, "bet": BETP, "onb": ONBa, "sel": SELa,
        })
    meta = (core_n, col_n)
    affine_id = bool(np.all(bet == 0.0) and np.all(gam == 1.0))
    return in_maps, tuple(int(x) for x in cis), affine_id, meta


def _assemble(results, meta):
    core_n, col_n = meta
    outs = np.stack([np.asarray(r["out"]) for r in results]).astype(np.float32)
    full = outs[core_n, :, col_n]          # [n_nodes, D]
    return np.ascontiguousarray(full)


# --------------------------------------------------------------------------
# public entry point
# --------------------------------------------------------------------------

_AXON_SO = "/opt/axon/libaxon_pjrt.so"


def _ensure_ntff_hook():
    """Provide antenv.axon_hooks + register the ctypes NTFF profile hook
    (the agent image's antenv lacks axon_hooks, so boot degraded silently)."""
    import sys
    import types
    import ctypes
    import contextlib
    import os

    try:
        from antenv.axon_hooks import get_axon_ntff_profile_hook  # noqa: F401
        return
    except ImportError:
        pass
    import antenv

    m = types.ModuleType("antenv.axon_hooks")
    m._hook = None

    def set_axon_ntff_profile_hook(h):
        m._hook = h

    def get_axon_ntff_profile_hook():
        return m._hook

    m.set_axon_ntff_profile_hook = set_axon_ntff_profile_hook
    m.get_axon_ntff_profile_hook = get_axon_ntff_profile_hook
    sys.modules["antenv.axon_hooks"] = m
    antenv.axon_hooks = m

    if not os.path.exists(_AXON_SO):
        return
    lib = ctypes.CDLL(_AXON_SO)
    if not hasattr(lib, "axon_start_nrt_profile"):
        return
    lib.axon_start_nrt_profile.argtypes = [ctypes.POINTER(ctypes.c_int64),
                                           ctypes.c_size_t]
    lib.axon_start_nrt_profile.restype = ctypes.c_int64
    lib.axon_stop_nrt_profile.argtypes = [ctypes.c_char_p]
    lib.axon_stop_nrt_profile.restype = ctypes.c_int64

    @contextlib.contextmanager
    def _hook(output_dir, device_ids):
        import jax

        jax.devices()
        if device_ids:
            ids = (ctypes.c_int64 * len(device_ids))(*device_ids)
            rc = lib.axon_start_nrt_profile(ids, len(device_ids))
        else:
            rc = lib.axon_start_nrt_profile(None, 0)
        if rc != 0:
            raise RuntimeError(f"axon_start_nrt_profile rc={rc}")
        try:
            yield
        finally:
            n = lib.axon_stop_nrt_profile(str(output_dir).encode())
            if n < 0:
                raise RuntimeError(f"axon_stop_nrt_profile rc={n}")
            if n == 0:
                print("WARNING: NTFF capture wrote no files")

    m._hook = _hook


def _run(inputs, trace=False):
    if trace:
        _ensure_ntff_hook()
    in_maps, cis, affine_id, meta = _preprocess(inputs)
    nc = _build(cis, N_CORES, affine_id)
    res = bass_utils.run_bass_kernel_spmd(
        nc, in_maps, core_ids=list(range(N_CORES)), trace=trace)
    out = _assemble(res.results, meta)
    return out, res


def kernel(**inputs):
    out, _ = _run(inputs, trace=False)
    return out


def kernel_profiled(**inputs):
    out, res = _run(inputs, trace=True)
    return out, res
